# revision 8
# baseline (speedup 1.0000x reference)
"""Trainium2 Bass kernel for nn_GeneSetPlaceholderAggregator.

Computes out[b,s,d] = sum_g x[b,g,d] * W[s,g]  (einsum 'bgd,sg->bsd')
with B=64, G=20000, D=16, S=128.

Strategy: shard the contraction axis G across 8 cores (2500 genes each).
Each core computes a full partial output [S=128, B*D=1024] via PSUM-
accumulated matmuls (contraction on the partition dim), and the host sums
the 8 partials.  Host pre-transposes x -> [G, B*D] and W -> [G, S] so
every DMA is a contiguous block.  Per-core traffic: 10 MB x-shard +
1.25 MB W-shard + 0.5 MB out, vs 20.5 MB for batch-parallel sharding.
"""

import numpy as np

import concourse.mybir as mybir
from concourse import bass
from concourse.bacc import Bacc
from concourse.bass_utils import run_bass_kernel_spmd
from concourse.tile import TileContext

B, G, D, S = 64, 20000, 16, 128
N_CORES = 8
G_LOC = G // N_CORES          # 2500 genes per core
K_CHUNK = 125                 # contraction-tile partition size
N_CHUNKS = G_LOC // K_CHUNK   # 20
BD = B * D                    # 1024
FREE = 512                    # max fp32 free dim per PSUM bank / matmul
N_FREE = BD // FREE           # 2

MM_DT = mybir.dt.float32


ROW = BD + S                  # 1152: [x row | w row] packed per gene


def build_nc() -> bass.Bass:
    nc = Bacc("TRN2", target_bir_lowering=False)

    xw = nc.declare_dram_parameter("xw", [G_LOC, ROW], mybir.dt.float32, isOutput=False)
    out = nc.declare_dram_parameter("out", [S, BD], mybir.dt.float32, isOutput=True)

    with TileContext(nc) as tc:
        with (
            tc.tile_pool(name="xp", bufs=4) as xp,
            tc.tile_pool(name="op", bufs=2) as op,
            tc.tile_pool(name="ps", bufs=N_FREE, space="PSUM") as ps,
        ):
            psums = [
                ps.tile([S, FREE], mybir.dt.float32, name=f"psum{j}")
                for j in range(N_FREE)
            ]
            for c in range(N_CHUNKS):
                xw_t = xp.tile([K_CHUNK, ROW], mybir.dt.float32)
                nc.sync.dma_start(out=xw_t[:], in_=xw[c * K_CHUNK:(c + 1) * K_CHUNK, :])
                for j in range(N_FREE):
                    nc.tensor.matmul(
                        psums[j][:],
                        lhsT=xw_t[:, BD:ROW].bitcast(MM_DT),
                        rhs=xw_t[:, j * FREE:(j + 1) * FREE].bitcast(MM_DT),
                        start=(c == 0),
                        stop=(c == N_CHUNKS - 1),
                    )
            for j in range(N_FREE):
                o_t = op.tile([S, FREE], mybir.dt.float32)
                nc.scalar.copy(out=o_t[:], in_=psums[j][:])
                nc.sync.dma_start(out=out[:, j * FREE:(j + 1) * FREE], in_=o_t[:])
    nc.compile()
    return nc


_CACHE: dict = {}


def _get_nc() -> bass.Bass:
    if "nc" not in _CACHE:
        _CACHE["nc"] = build_nc()
    return _CACHE["nc"]


def _shard_inputs(x: np.ndarray, W: np.ndarray) -> list[dict[str, np.ndarray]]:
    # Pack per-gene rows [x[:, g, :].ravel() | W[:, g]] -> XW [G, B*D + S]
    XW = np.empty((G, ROW), dtype=np.float32)
    XW[:, :BD] = x.transpose(1, 0, 2).reshape(G, BD)
    XW[:, BD:] = W.T
    return [{"xw": XW[i * G_LOC:(i + 1) * G_LOC]} for i in range(N_CORES)]


def run(x: np.ndarray, W: np.ndarray, **spmd_kwargs):
    nc = _get_nc()
    in_maps = _shard_inputs(x, W)
    res = run_bass_kernel_spmd(nc, in_maps, list(range(N_CORES)), **spmd_kwargs)
    partial = np.zeros((S, BD), dtype=np.float64)
    for r in res.results:
        partial += r["out"].astype(np.float64)
    out = partial.astype(np.float32).reshape(S, B, D).transpose(1, 0, 2)
    return np.ascontiguousarray(out), res


def kernel(x: np.ndarray, W: np.ndarray) -> np.ndarray:
    out, _ = run(x, W)
    return out
